# revision 54
# baseline (speedup 1.0000x reference)
"""Trainium2 Bass kernel: causal multi-head attention with RoPE (pipelined v3).

Problem: B=2, T=2048, C=1024, H=16, HD=64.
  q/k/v = x @ W{q,k,v}.T ; rope(q), rope(k)
  att = softmax(causal(q k^T / 8)) ; out = (att v) @ Wo.T

Sharding (8 cores): core i handles batch b = i//4 and head group g = i%4
(4 heads = 2 head-pairs). Each core computes its partial output
x[b]-slice @ Wo[:, slice].T; the host sums the 4 bf16 partials per batch.

Schedule: single software-pipelined pass.  The attention k-tile loop
(ScalarE-exp-bound, ~1.1us/tile) is the backbone; all other PE work (QKV
projection chunks, V staging, output projection) is emitted as "filler"
closures popped between k-tiles so TensorE never idles while ScalarE
runs exp.  The inner loop is software-pipelined by one k-tile (scores of
tile t+1 are emitted before att@V of tile t) so a blocked att@V does not
head-of-line-block independent matmuls on the in-order PE queue.
ScalarE does exp ONLY; PSUM->SBUF copies live on VectorE.
Input tensors are staged as single wide SBUF tiles so each input needs
ONE multi-dim-AP DMA (DMA-issue instructions cost ~0.6us of queue time).
PSUM: 2x score buffers (2 banks each), 1 attV accumulator (2 banks,
released early via one fp32->bf16 CAST of the whole accumulator),
2x 1-bank filler buffers for QKV/proj matmuls.
Diagonal score tiles pack head B's valid columns right after head A's
so exp processes no masked garbage.  Output partials are bf16.
"""

import os
from collections import deque

import numpy as np
import ml_dtypes

B, T, C, H, HD = 2, 2048, 1024, 16, 64
N_CORES = 8
GROUPS = 4  # head groups (of 4 heads) per batch
HPG = H // GROUPS  # heads per core = 4
M_CORE = HPG * HD  # 256 head channels per core
PAIRS = HPG // 2  # head pairs per core = 2
QCHUNK = 512  # q columns per attention chunk
KTILE = 128  # k rows per tile
NQC = T // QCHUNK  # 4
NT128 = T // 128  # 16
NCT = C // 128  # 8 contraction tiles

_bf16 = ml_dtypes.bfloat16

_CACHE = {}
LAST_RESULTS = None  # BassKernelResults of the most recent run (for test.py)


def _build_bass():
    """Trace the per-core Bass/Tile program (SPMD, same NEFF on all cores)."""
    from contextlib import ExitStack

    import concourse.bass as bass
    import concourse.tile as tile
    from concourse import bacc, mybir

    f32 = mybir.dt.float32
    bf16 = mybir.dt.bfloat16
    Exp = mybir.ActivationFunctionType.Exp

    nc = bacc.Bacc(
        "TRN2",
        target_bir_lowering=False,
        debug=False,
        enable_asserts=False,
        num_devices=N_CORES,
    )

    # all inputs host-pre-shuffled to [128, *] so every load is a contiguous
    # big-line DMA (DMA-issue cost scales with descriptor-line count)
    xt_d = nc.dram_tensor("xt", [128, NQC * NCT * QCHUNK], bf16, kind="ExternalInput").ap()
    wq_d = nc.dram_tensor("wqt", [128, NCT * M_CORE], bf16, kind="ExternalInput").ap()
    wk_d = nc.dram_tensor("wkt", [128, NCT * M_CORE], bf16, kind="ExternalInput").ap()
    wv_d = nc.dram_tensor("wvt", [128, NCT * M_CORE], bf16, kind="ExternalInput").ap()
    wo_d = nc.dram_tensor("wot", [128, PAIRS * C], bf16, kind="ExternalInput").ap()
    cmap_d = nc.dram_tensor("cmap", [128, T], bf16, kind="ExternalInput").ap()
    smap_d = nc.dram_tensor("smap", [128, T], bf16, kind="ExternalInput").ap()
    psh_d = nc.dram_tensor("pshift", [128, 128], bf16, kind="ExternalInput").ap()
    psw_d = nc.dram_tensor("pswap", [128, 128], bf16, kind="ExternalInput").ap()
    out_d = nc.dram_tensor("out", [T, C], bf16, kind="ExternalOutput").ap()

    with tile.TileContext(nc) as tc:
        with ExitStack() as ctx:
            consts = ctx.enter_context(tc.tile_pool(name="consts", bufs=1))
            qk_sb = ctx.enter_context(tc.tile_pool(name="qk_sb", bufs=1))
            rope_tmp = ctx.enter_context(tc.tile_pool(name="rope_tmp", bufs=3))
            att_sb = ctx.enter_context(tc.tile_pool(name="att_sb", bufs=8))
            misc_sb = ctx.enter_context(tc.tile_pool(name="misc_sb", bufs=2))
            out_sb = ctx.enter_context(tc.tile_pool(name="out_sb", bufs=3))
            ps_st = ctx.enter_context(
                tc.tile_pool(name="ps_st", bufs=2, space="PSUM")
            )
            ps_os = ctx.enter_context(
                tc.tile_pool(name="ps_os", bufs=1, space="PSUM")
            )
            ps_fl = ctx.enter_context(
                tc.tile_pool(name="ps_fl", bufs=2, space="PSUM")
            )

            # ---- wide staging tiles: ONE DMA per input tensor/chunk ----
            xtall = consts.tile([128, NQC * NCT * QCHUNK], bf16, tag="xtall", name="xtall")
            wqall = consts.tile([128, NCT * M_CORE], bf16, tag="wqall", name="wqall")
            wkall = consts.tile([128, NCT * M_CORE], bf16, tag="wkall", name="wkall")
            wvall = consts.tile([128, NCT * M_CORE], bf16, tag="wvall", name="wvall")
            woall = consts.tile([128, PAIRS * C], bf16, tag="woall", name="woall")
            cmap = consts.tile([128, T], bf16, tag="cmap", name="cmap")
            smap = consts.tile([128, T], bf16, tag="smap", name="smap")
            tri = consts.tile([128, 128], bf16, tag="tri", name="tri")
            psh = consts.tile([128, 128], bf16, tag="psh", name="psh")
            psw = consts.tile([128, 128], bf16, tag="psw", name="psw")

            # xtall layout: chunk-major [c][ci][512 cols]
            def xt_ap(ci, c0, c1):
                c = c0 // QCHUNK
                base = c * (NCT * QCHUNK) + ci * QCHUNK + (c0 - c * QCHUNK)
                return xtall[:, base : base + (c1 - c0)]

            def w_ap(w, ci, p):  # [128,128] pair-p slice of c-tile ci
                # wq/wk host layout is pair-major: [pair][ci][128]
                return w[:, p * C + ci * 128 : p * C + (ci + 1) * 128]

            def wv_ap(ci):
                return wvall[:, ci * M_CORE : (ci + 1) * M_CORE]

            def wo_ap(p, c0, c1):
                return woall[:, p * C + c0 : p * C + c1]

            def load_xt_chunk(eng, c):
                cs = slice(c * NCT * QCHUNK, (c + 1) * NCT * QCHUNK)
                eng.dma_start(xtall[:, cs], xt_d[:, cs])

            qt_r = [qk_sb.tile([128, T], bf16, tag=f"qtr{p}", name=f"qtr{p}") for p in range(PAIRS)]
            kt_r = [qk_sb.tile([128, T], bf16, tag=f"ktr{p}", name=f"ktr{p}") for p in range(PAIRS)]
            v_ext = [qk_sb.tile([128, 4 * 128], bf16, tag=f"v{tt}", name=f"v{tt}") for tt in range(NT128)]
            att_out = [qk_sb.tile([128, T], bf16, tag=f"ao{p}", name=f"ao{p}") for p in range(PAIRS)]

            # ---- input DMAs: ALL on the scalar ring (ACT idle early) so
            # transfers serialize in need-order -- concurrent rings would
            # steal HBM bandwidth from the critical q0c0/k0c0 loads ----
            nc.scalar.dma_start(wqall[:, 0:C], wq_d[:, 0:C])  # pair-0 wq
            nc.scalar.dma_start(psh[:], psh_d[:])             # rope shift perm
            nc.scalar.dma_start(psw[:], psw_d[:])             # 64-half swap perm
            half = NCT * QCHUNK // 2
            nc.scalar.dma_start(xtall[:, 0:half], xt_d[:, 0:half])
            nc.scalar.dma_start(xtall[:, half : 2 * half], xt_d[:, half : 2 * half])
            nc.scalar.dma_start(wkall[:, 0:C], wk_d[:, 0:C])  # pair-0 wk
            nc.scalar.dma_start(cmap[:], cmap_d[:])
            nc.scalar.dma_start(smap[:], smap_d[:])
            nc.scalar.dma_start(wvall[:], wv_d[:])
            load_xt_chunk(nc.scalar, 1)
            nc.scalar.dma_start(wqall[:, C:], wq_d[:, C:])    # pair-1 wq
            nc.scalar.dma_start(wkall[:, C:], wk_d[:, C:])    # pair-1 wk
            load_xt_chunk(nc.scalar, 2)
            load_xt_chunk(nc.scalar, 3)
            nc.scalar.dma_start(woall[:], wo_d[:])

            # upper-triangular (incl. diagonal) keep-mask: tri[p, y] = p <= y
            nc.gpsimd.memset(tri[:], 1.0)
            nc.gpsimd.affine_select(
                out=tri[:],
                in_=tri[:],
                compare_op=mybir.AluOpType.is_ge,
                fill=0.0,
                base=0,
                pattern=[[1, 128]],
                channel_multiplier=-1,
            )
            tri_b = tri[:]  # broadcast view over 2 mask blocks built per-use

            # ---- PE warm-up: spin matmuls on tri during the input-DMA wait
            # so q0c0 runs at 2.4 GHz (HAM un-throttles after ~3.4us busy)
            wps = ps_fl.tile([128, 128], f32, tag="fl", name="ps_warm")
            for _ in range(46):
                nc.tensor.matmul(wps[:], lhsT=tri[:], rhs=tri[:], start=True, stop=True)

            # ---- work units -------------------------------------------------
            emitted = set()

            def qk_units(which, wall, p, c, dst, fast=False):
                """q or k chunk as 5 fine units: 4x(2 matmuls), then rope."""
                cs = slice(c * QCHUNK, (c + 1) * QCHUNK)
                st = {}

                def mms(u):
                    def fn():
                        if u == 0:
                            st["ps"] = ps_fl.tile([128, QCHUNK], f32, tag="fl", name="ps_qk")
                        for ci in (2 * u, 2 * u + 1):
                            nc.tensor.matmul(
                                st["ps"][:],
                                lhsT=w_ap(wall, ci, p),
                                rhs=xt_ap(ci, c * QCHUNK, (c + 1) * QCHUNK),
                                start=(ci == 0),
                                stop=(ci == NCT - 1),
                            )
                    return fn

                def rope():
                    raw = rope_tmp.tile([128, QCHUNK], bf16, tag="raw", name="raw")
                    nc.vector.tensor_copy(raw[:], st["ps"][:])
                    t1 = rope_tmp.tile([128, QCHUNK], bf16, tag="t1", name="t1")
                    nc.vector.tensor_mul(t1[:], raw[:], cmap[:, cs])
                    t2 = rope_tmp.tile([128, QCHUNK], bf16, tag="t2", name="t2")
                    # swap 32-row halves within each 64-row head block via a
                    # PE permutation matmul (DMA/GpSimd shifts both have
                    # us-scale latency or throughput problems)
                    shp = ps_fl.tile([128, QCHUNK], f32, tag="fl", name="ps_shf")
                    nc.tensor.matmul(
                        shp[:], lhsT=psh[:], rhs=raw[:], start=True, stop=True
                    )
                    nc.vector.tensor_mul(t2[:], shp[:], smap[:, cs])
                    nc.vector.tensor_add(dst[:, cs], t1[:], t2[:])
                    emitted.add((which, p, c))

                return [(450, mms(0)), (430, mms(1)), (430, mms(2)), (430, mms(3)),
                        (260 if fast else 60, rope)]

            def v_units(tt):
                """V for t-tile tt: 2x(4 matmuls) + interleave copies."""
                st = {}

                def mms(u):
                    def fn():
                        if u == 0:
                            nc.gpsimd.memset(v_ext[tt][:], 1.0)
                            st["ps"] = ps_fl.tile([128, M_CORE], f32, tag="fl", name="ps_v")
                        for ci in range(4 * u, 4 * u + 4):
                            nc.tensor.matmul(
                                st["ps"][:],
                                lhsT=xt_ap(ci, tt * 128, (tt + 1) * 128),
                                rhs=wv_ap(ci),
                                start=(ci == 0),
                                stop=(ci == NCT - 1),
                            )
                    return fn

                def copies():
                    vt, ps = v_ext[tt], st["ps"]
                    # dst layout: [V0 | 1 .. 1 | V1][V2 | 1 .. 1 | V3]
                    nc.vector.tensor_copy(vt[:, 0:64], ps[:, 0:64])
                    nc.vector.tensor_copy(vt[:, 192:320], ps[:, 64:192])
                    nc.vector.tensor_copy(vt[:, 448:512], ps[:, 192:256])
                    emitted.add(("v", tt))

                return [(460, mms(0)), (430, mms(1)), (60, copies)]

            def proj_units(qt, tailpool=False):
                """Output projection for q-tile qt: [128,1024] fp32->bf16->HBM."""
                st = {}
                # the last group drains after attention: the score-psum banks
                # are free then, doubling the number of concurrent proj chains
                pool, ptag = (ps_st, "st") if tailpool else (ps_fl, "fl")

                def half(jc):
                    def fn():
                        if jc == 0:
                            st["ob"] = out_sb.tile([128, C], bf16, tag="ob", name="ob")
                        ps = pool.tile([128, QCHUNK], f32, tag=ptag, name="ps_pj")
                        for p in range(PAIRS):
                            nc.tensor.matmul(
                                ps[:],
                                lhsT=att_out[p][:, qt * 128 : (qt + 1) * 128],
                                rhs=wo_ap(p, jc * QCHUNK, (jc + 1) * QCHUNK),
                                start=(p == 0),
                                stop=(p == PAIRS - 1),
                            )
                        nc.vector.tensor_copy(
                            st["ob"][:, jc * QCHUNK : (jc + 1) * QCHUNK], ps[:]
                        )
                        if jc == 1:
                            oeng = nc.sync if qt % 2 == 0 else nc.gpsimd
                            oeng.dma_start(
                                out_d[qt * 128 : (qt + 1) * 128, :], st["ob"][:]
                            )
                    return fn

                return [(460, half(0)), (460, half(1))]

            # ---- filler queue (self-balancing pop budget) ----
            fillq = deque()  # (pe_cost_ns, closure)
            debt = [0.0]
            pops_left = [60]  # pop points, deflated: cost estimates undershoot

            proj_pending = [14720]  # cost of proj units not yet appended

            def pop_fill():
                queued = sum(c for c, _ in fillq) + proj_pending[0]
                budget = max(500.0, min(1500.0, queued / max(pops_left[0], 1)))
                pops_left[0] -= 1
                debt[0] += budget
                while fillq and debt[0] > 0:
                    cost, fn = fillq.popleft()
                    fn()
                    debt[0] -= cost

            def force_until(labels):
                while any(l not in emitted for l in labels):
                    assert fillq, f"missing prereqs {labels}"
                    cost, fn = fillq.popleft()
                    fn()
                    debt[0] -= cost

            # ---- attention chunk (inner loop software-pipelined by 1) ----
            def attn_chunk(p, j, next_qk=None, last=False):
                force_until([("q", p, j)] + [("k", p, c) for c in range(j + 1)])
                os2 = ps_os.tile([128, 2 * QCHUNK], f32, tag="os", name="ps_os")
                outA = os2[:, 0:QCHUNK]   # rows 0:64 attV_A, 64:128 sums_A
                outB = os2[:, QCHUNK:]    # rows 0:64 sums_B, 64:128 attV_B
                nkt = (j + 1) * (QCHUNK // KTILE)
                atts = [None] * nkt  # att2 tile + c0 per kb, for deferred AV

                def emit_scores(kb):
                    o = KTILE * kb - QCHUNK * j
                    c0 = max(o, 0)
                    qs = slice(j * QCHUNK + c0, (j + 1) * QCHUNK)
                    ks = slice(kb * KTILE, (kb + 1) * KTILE)
                    # both heads' scores in one 2-bank tile -> single exp;
                    # head B packed at column QCHUNK (not QCHUNK+c0) so the
                    # exp span [c0, 2*QCHUNK-c0) has no garbage columns
                    st2 = ps_st.tile([128, 2 * QCHUNK], f32, tag="st", name="ps_st")
                    nc.tensor.matmul(
                        st2[:, c0:QCHUNK],
                        lhsT=kt_r[p][0:64, ks],
                        rhs=qt_r[p][0:64, qs],
                        start=True,
                        stop=True,
                        tile_position=(0, 0),
                    )
                    nc.tensor.matmul(
                        st2[:, QCHUNK : 2 * QCHUNK - c0],
                        lhsT=kt_r[p][64:128, ks],
                        rhs=qt_r[p][64:128, qs],
                        start=True,
                        stop=True,
                        tile_position=(64, 0),
                    )
                    att2 = att_sb.tile([128, 2 * QCHUNK], bf16, tag="att", name="att2")
                    nc.scalar.activation(
                        att2[:, c0 : 2 * QCHUNK - c0],
                        st2[:, c0 : 2 * QCHUNK - c0],
                        Exp,
                        scale=0.125,
                    )
                    if o >= 0:  # diagonal tile: triangular mask, both heads
                        blk = QCHUNK - o
                        a = att2[:]
                        m_ap = bass.AP(a.tensor, a.offset + o, [list(a.ap[0]), [blk, 2], [1, 128]])
                        t_ap = bass.AP(tri_b.tensor, tri_b.offset, [list(tri_b.ap[0]), [0, 2], [1, 128]])
                        nc.vector.tensor_mul(m_ap, m_ap, t_ap)
                    atts[kb] = (att2, c0)

                def emit_av(kb):
                    att2, c0 = atts[kb]
                    atts[kb] = None
                    start = kb == 0
                    stop = kb == nkt - 1
                    blkA = slice((2 * p) * 128, (2 * p) * 128 + 128)
                    blkB = slice((2 * p + 1) * 128, (2 * p + 1) * 128 + 128)
                    nc.tensor.matmul(
                        outA[:, c0:],
                        lhsT=v_ext[kb][:, blkA],
                        rhs=att2[:, c0:QCHUNK],
                        start=start,
                        stop=stop,
                    )
                    nc.tensor.matmul(
                        outB[:, c0:],
                        lhsT=v_ext[kb][:, blkB],
                        rhs=att2[:, QCHUNK : 2 * QCHUNK - c0],
                        start=start,
                        stop=stop,
                    )

                for kb in range(nkt):
                    force_until([("v", kb)])
                    emit_scores(kb)
                    if kb == nkt // 2 and next_qk:
                        # prefetch next chunk's q/k so its rope latency
                        # hides under this chunk's exp stream
                        force_until(next_qk)
                    if kb > 0:
                        if not (last and kb > nkt - 3):
                            # keep the tail lean: no filler DVE work queued
                            # ahead of the final normalize chain
                            pop_fill()
                        emit_av(kb - 1)
                emit_av(nkt - 1)

                # release the accumulator early: one whole-tile cast to SBUF
                osb = misc_sb.tile([128, 2 * QCHUNK], bf16, tag="osb", name="osb")
                nc.vector.tensor_copy(osb[:], os2[:])
                oA = osb[:, 0:QCHUNK]
                oB = osb[:, QCHUNK:]
                cs = slice(j * QCHUNK, (j + 1) * QCHUNK)
                if last:
                    # latency-critical final normalize: 64-half swap on PE
                    # FIRST (all-finite bf16 operand -- recip-then-swap would
                    # feed inf junk lanes into the matmul and 0*inf=NaN),
                    # then reciprocal; junk lanes never enter any matmul
                    r0 = ps_fl.tile([128, QCHUNK], f32, tag="fl", name="ps_r0")
                    nc.tensor.matmul(r0[:], lhsT=psw[:], rhs=oA, start=True, stop=True)
                    r1 = ps_fl.tile([128, QCHUNK], f32, tag="fl", name="ps_r1")
                    nc.tensor.matmul(r1[:], lhsT=psw[:], rhs=oB, start=True, stop=True)
                    recA = misc_sb.tile([128, QCHUNK], f32, tag="recA", name="recA")
                    nc.vector.reciprocal_approx_fast(recA[:], r0[:])
                    recB = misc_sb.tile([128, QCHUNK], f32, tag="recB", name="recB")
                    nc.vector.reciprocal_approx_fast(recB[:], r1[:])
                    nc.vector.tensor_mul(att_out[p][0:64, cs], oA[0:64, :], recA[0:64, :])
                    nc.vector.tensor_mul(att_out[p][64:128, cs], oB[64:128, :], recB[64:128, :])
                    return
                # gather sums (aligned sub-partition copies), one reciprocal:
                # rows 0:64 = 1/sums_B, rows 64:128 = 1/sums_A
                sc = misc_sb.tile([128, QCHUNK], f32, tag="sc", name="sums_sb")
                nc.vector.tensor_copy(sc[0:64, :], oB[0:64, :])
                nc.vector.tensor_copy(sc[64:128, :], oA[64:128, :])
                rec_raw = misc_sb.tile([128, QCHUNK], f32, tag="rec_raw", name="rec_raw")
                nc.vector.reciprocal_approx_fast(rec_raw[:], sc[:])
                # swap halves so divisors align with their heads' rows
                rec = misc_sb.tile([128, QCHUNK], f32, tag="rec", name="rec")
                deng = nc.gpsimd if p == 0 else nc.sync
                deng.dma_start(rec[0:64, :], rec_raw[64:128, :])
                deng.dma_start(rec[64:128, :], rec_raw[0:64, :])
                nc.vector.tensor_mul(att_out[p][0:64, cs], oA[0:64, :], rec[0:64, :])
                nc.vector.tensor_mul(att_out[p][64:128, cs], oB[64:128, :], rec[64:128, :])

            # ---- prologue: minimum needed for attn(0,0) ----
            for _cost, fn in (
                qk_units("q", wqall, 0, 0, qt_r[0], fast=True)
                + qk_units("k", wkall, 0, 0, kt_r[0], fast=True)
                + v_units(0)
            ):
                fn()

            # ---- queue the rest, in exact deadline order:
            # V tiles of group c are needed (per-kb) before chunk (p,c) ends;
            # q/k of (1,c) at (1,c) start; q/k of (0,c+1) at (0,c+1) start --
            # which precedes (0,c+1)'s V group ----
            for tt in (1, 2, 3):
                fillq.extend(v_units(tt))
            fillq.extend(qk_units("q", wqall, 1, 0, qt_r[1], fast=True))
            fillq.extend(qk_units("k", wkall, 1, 0, kt_r[1], fast=True))
            for c in range(1, NQC):
                fillq.extend(qk_units("q", wqall, 0, c, qt_r[0], fast=True))
                fillq.extend(qk_units("k", wkall, 0, c, kt_r[0], fast=True))
                for tt in range(4 * c, 4 * c + 4):
                    fillq.extend(v_units(tt))
                fillq.extend(qk_units("q", wqall, 1, c, qt_r[1], fast=True))
                fillq.extend(qk_units("k", wkall, 1, c, kt_r[1], fast=True))

            # ---- main pipeline: alternate pairs so chunk boundaries of one
            # pair overlap the other pair's independent attention work ----
            order = [(p, j) for j in range(NQC) for p in range(PAIRS)]
            for idx, (p, j) in enumerate(order):
                nxt = order[idx + 1] if idx + 1 < len(order) else None
                next_qk = (
                    [("q", nxt[0], nxt[1]), ("k", nxt[0], nxt[1])] if nxt else None
                )
                attn_chunk(p, j, next_qk, last=(nxt is None))
                if p == 1:
                    for qt in range(4 * j, 4 * j + 4):
                        fillq.extend(proj_units(qt, tailpool=(j == NQC - 1)))
                    proj_pending[0] -= 3680
            while fillq:
                _c, fn = fillq.popleft()
                fn()

    nc.compile()
    return nc


def _prep_inputs(x, Wq, Wk, Wv, Wo, cos, sin):
    """Host-side sharding + layout prep. Returns list of per-core in_maps."""
    x = np.asarray(x, np.float32)
    Wq, Wk, Wv, Wo = (np.asarray(w, np.float32) for w in (Wq, Wk, Wv, Wo))
    cos, sin = np.asarray(cos, np.float32), np.asarray(sin, np.float32)

    # permute W rows to [evens; odds] within each head (rope pairing -> +-32)
    perm = np.concatenate(
        [
            np.concatenate(
                [np.arange(h * HD, (h + 1) * HD, 2), np.arange(h * HD + 1, (h + 1) * HD, 2)]
            )
            for h in range(H)
        ]
    )
    Wqp = Wq[perm]
    Wkp = Wk[perm]

    # rope maps [128, T] (identical for both heads of a pair, all cores)
    cosT = cos.T  # [32, T]
    sinT = sin.T
    cmap = np.empty((128, T), np.float32)
    smap = np.empty((128, T), np.float32)
    for blk in range(4):
        cmap[blk * 32 : (blk + 1) * 32] = cosT
        smap[blk * 32 : (blk + 1) * 32] = sinT if blk % 2 else -sinT
    cmap = cmap.astype(_bf16)
    smap = smap.astype(_bf16)

    # device layouts are [128, *] with big contiguous per-partition lines:
    # xt: [128, c(4) x ci(8) x 512]; wq/wk/wv: [128, ci(8) x 256];
    # wo: [128, pair(2) x 1024]
    def shuf_xt(xT):  # xT [C, T]
        v = xT.reshape(NCT, 128, NQC, QCHUNK)  # (ci, p, c, u)
        return np.ascontiguousarray(
            v.transpose(1, 2, 0, 3).reshape(128, NQC * NCT * QCHUNK)
        ).astype(_bf16)

    def shuf_w(wT):  # wT [C, M_CORE] -> [128, ci(8) x 256] (ci-major, for wv)
        v = wT.reshape(NCT, 128, M_CORE)  # (ci, p, v)
        return np.ascontiguousarray(
            v.transpose(1, 0, 2).reshape(128, NCT * M_CORE)
        ).astype(_bf16)

    def shuf_w_pair(wT):  # wT [C, M_CORE] -> [128, pair(2) x ci(8) x 128]
        v = wT.reshape(NCT, 128, PAIRS, 128)  # (ci, p, pair, col)
        return np.ascontiguousarray(
            v.transpose(1, 2, 0, 3).reshape(128, NCT * M_CORE)
        ).astype(_bf16)

    def shuf_wo(woT):  # woT [M_CORE, C]
        v = woT.reshape(PAIRS, 128, C)  # (pair, p, v)
        return np.ascontiguousarray(
            v.transpose(1, 0, 2).reshape(128, PAIRS * C)
        ).astype(_bf16)

    xTb = [shuf_xt(x[b].T) for b in range(B)]

    # rope shift permutation: out row j = in row j^32 (32-row half swap
    # within each 64-row head block), applied on PE as out = P.T @ raw
    psh = np.zeros((128, 128), np.float32)
    psh[np.arange(128), np.arange(128) ^ 32] = 1.0
    psh = psh.astype(_bf16)
    psw = np.zeros((128, 128), np.float32)
    psw[np.arange(128), np.arange(128) ^ 64] = 1.0
    psw = psw.astype(_bf16)

    in_maps = []
    for core in range(N_CORES):
        b, g = divmod(core, GROUPS)
        ms = slice(g * M_CORE, (g + 1) * M_CORE)
        in_maps.append(
            {
                "xt": xTb[b],
                "wqt": shuf_w_pair(Wqp[ms].T),
                "wkt": shuf_w_pair(Wkp[ms].T),
                "wvt": shuf_w(Wv[ms].T),
                "wot": shuf_wo(Wo[:, ms].T),
                "cmap": cmap,
                "smap": smap,
                "pshift": psh,
                "pswap": psw,
            }
        )
    return in_maps


def _ensure_ntff_hook():
    """Install an antenv.axon_hooks shim so trace=True works in this
    container (the image's antenv lacks the axon_hooks module)."""
    import sys
    import types

    try:
        from antenv.axon_hooks import get_axon_ntff_profile_hook  # noqa: F401

        return
    except ImportError:
        pass
    sys.path.insert(0, "/root/.axon_site")
    from trn_agent_boot.trn_boot import _ntff_profile_via_ctypes

    hook = _ntff_profile_via_ctypes("/opt/axon/libaxon_pjrt.so")
    mod = types.ModuleType("antenv.axon_hooks")
    mod._hook = hook
    mod.get_axon_ntff_profile_hook = lambda: mod._hook
    mod.set_axon_ntff_profile_hook = lambda h: setattr(mod, "_hook", h)
    sys.modules["antenv.axon_hooks"] = mod

    # no bucket creds in this container; keep artifacts local
    import concourse.bass_utils as bu

    bu.upload_artifacts = lambda tmpdir: tmpdir


def kernel(x, Wq, Wk, Wv, Wo, cos, sin):
    global LAST_RESULTS
    from concourse.bass_utils import run_bass_kernel_spmd

    if "nc" not in _CACHE:
        _CACHE["nc"] = _build_bass()
    nc = _CACHE["nc"]

    in_maps = _prep_inputs(x, Wq, Wk, Wv, Wo, cos, sin)
    trace = bool(int(os.environ.get("KERNEL_TRACE", "0")))
    if trace:
        _ensure_ntff_hook()
    res = run_bass_kernel_spmd(
        nc, in_maps, core_ids=list(range(N_CORES)), trace=trace
    )
    LAST_RESULTS = res

    out = np.zeros((B, T, C), np.float32)
    for core in range(N_CORES):
        b = core // GROUPS
        out[b] += res.results[core]["out"].astype(np.float32)
    return out


# revision 55
# speedup vs baseline: 1.0103x; 1.0103x over previous
"""Trainium2 Bass kernel: causal multi-head attention with RoPE (pipelined v3).

Problem: B=2, T=2048, C=1024, H=16, HD=64.
  q/k/v = x @ W{q,k,v}.T ; rope(q), rope(k)
  att = softmax(causal(q k^T / 8)) ; out = (att v) @ Wo.T

Sharding (8 cores): core i handles batch b = i//4 and head group g = i%4
(4 heads = 2 head-pairs). Each core computes its partial output
x[b]-slice @ Wo[:, slice].T; the host sums the 4 bf16 partials per batch.

Schedule: single software-pipelined pass.  The attention k-tile loop
(ScalarE-exp-bound, ~1.1us/tile) is the backbone; all other PE work (QKV
projection chunks, V staging, output projection) is emitted as "filler"
closures popped between k-tiles so TensorE never idles while ScalarE
runs exp.  The inner loop is software-pipelined by one k-tile (scores of
tile t+1 are emitted before att@V of tile t) so a blocked att@V does not
head-of-line-block independent matmuls on the in-order PE queue.
ScalarE does exp ONLY; PSUM->SBUF copies live on VectorE.
Input tensors are staged as single wide SBUF tiles so each input needs
ONE multi-dim-AP DMA (DMA-issue instructions cost ~0.6us of queue time).
PSUM: 2x score buffers (2 banks each), 1 attV accumulator (2 banks,
released early via one fp32->bf16 CAST of the whole accumulator),
2x 1-bank filler buffers for QKV/proj matmuls.
Diagonal score tiles pack head B's valid columns right after head A's
so exp processes no masked garbage.  Output partials are bf16.
"""

import os
from collections import deque

import numpy as np
import ml_dtypes

B, T, C, H, HD = 2, 2048, 1024, 16, 64
N_CORES = 8
GROUPS = 4  # head groups (of 4 heads) per batch
HPG = H // GROUPS  # heads per core = 4
M_CORE = HPG * HD  # 256 head channels per core
PAIRS = HPG // 2  # head pairs per core = 2
QCHUNK = 512  # q columns per attention chunk
KTILE = 128  # k rows per tile
NQC = T // QCHUNK  # 4
NT128 = T // 128  # 16
NCT = C // 128  # 8 contraction tiles

_bf16 = ml_dtypes.bfloat16

_CACHE = {}
LAST_RESULTS = None  # BassKernelResults of the most recent run (for test.py)


def _build_bass():
    """Trace the per-core Bass/Tile program (SPMD, same NEFF on all cores)."""
    from contextlib import ExitStack

    import concourse.bass as bass
    import concourse.tile as tile
    from concourse import bacc, mybir

    f32 = mybir.dt.float32
    bf16 = mybir.dt.bfloat16
    Exp = mybir.ActivationFunctionType.Exp

    nc = bacc.Bacc(
        "TRN2",
        target_bir_lowering=False,
        debug=False,
        enable_asserts=False,
        num_devices=N_CORES,
    )

    # all inputs host-pre-shuffled to [128, *] so every load is a contiguous
    # big-line DMA (DMA-issue cost scales with descriptor-line count)
    xt_d = nc.dram_tensor("xt", [128, NQC * NCT * QCHUNK], bf16, kind="ExternalInput").ap()
    wq_d = nc.dram_tensor("wqt", [128, NCT * M_CORE], bf16, kind="ExternalInput").ap()
    wk_d = nc.dram_tensor("wkt", [128, NCT * M_CORE], bf16, kind="ExternalInput").ap()
    wv_d = nc.dram_tensor("wvt", [128, NCT * M_CORE], bf16, kind="ExternalInput").ap()
    wo_d = nc.dram_tensor("wot", [128, PAIRS * C], bf16, kind="ExternalInput").ap()
    cmap_d = nc.dram_tensor("cmap", [128, T], bf16, kind="ExternalInput").ap()
    smap_d = nc.dram_tensor("smap", [128, T], bf16, kind="ExternalInput").ap()
    psh_d = nc.dram_tensor("pshift", [128, 128], bf16, kind="ExternalInput").ap()
    psw_d = nc.dram_tensor("pswap", [128, 128], bf16, kind="ExternalInput").ap()
    out_d = nc.dram_tensor("out", [T, C], bf16, kind="ExternalOutput").ap()

    with tile.TileContext(nc) as tc:
        with ExitStack() as ctx:
            consts = ctx.enter_context(tc.tile_pool(name="consts", bufs=1))
            qk_sb = ctx.enter_context(tc.tile_pool(name="qk_sb", bufs=1))
            rope_tmp = ctx.enter_context(tc.tile_pool(name="rope_tmp", bufs=3))
            att_sb = ctx.enter_context(tc.tile_pool(name="att_sb", bufs=8))
            misc_sb = ctx.enter_context(tc.tile_pool(name="misc_sb", bufs=2))
            out_sb = ctx.enter_context(tc.tile_pool(name="out_sb", bufs=3))
            ps_st = ctx.enter_context(
                tc.tile_pool(name="ps_st", bufs=2, space="PSUM")
            )
            ps_os = ctx.enter_context(
                tc.tile_pool(name="ps_os", bufs=1, space="PSUM")
            )
            ps_fl = ctx.enter_context(
                tc.tile_pool(name="ps_fl", bufs=2, space="PSUM")
            )

            # ---- wide staging tiles: ONE DMA per input tensor/chunk ----
            xtall = consts.tile([128, NQC * NCT * QCHUNK], bf16, tag="xtall", name="xtall")
            wqall = consts.tile([128, NCT * M_CORE], bf16, tag="wqall", name="wqall")
            wkall = consts.tile([128, NCT * M_CORE], bf16, tag="wkall", name="wkall")
            wvall = consts.tile([128, NCT * M_CORE], bf16, tag="wvall", name="wvall")
            woall = consts.tile([128, PAIRS * C], bf16, tag="woall", name="woall")
            cmap = consts.tile([128, T], bf16, tag="cmap", name="cmap")
            smap = consts.tile([128, T], bf16, tag="smap", name="smap")
            tri = consts.tile([128, 128], bf16, tag="tri", name="tri")
            psh = consts.tile([128, 128], bf16, tag="psh", name="psh")
            psw = consts.tile([128, 128], bf16, tag="psw", name="psw")

            # xtall layout: chunk-major [c][ci][512 cols]
            def xt_ap(ci, c0, c1):
                c = c0 // QCHUNK
                base = c * (NCT * QCHUNK) + ci * QCHUNK + (c0 - c * QCHUNK)
                return xtall[:, base : base + (c1 - c0)]

            def w_ap(w, ci, p):  # [128,128] pair-p slice of c-tile ci
                # wq/wk host layout is pair-major: [pair][ci][128]
                return w[:, p * C + ci * 128 : p * C + (ci + 1) * 128]

            def wv_ap(ci):
                return wvall[:, ci * M_CORE : (ci + 1) * M_CORE]

            def wo_ap(p, c0, c1):
                return woall[:, p * C + c0 : p * C + c1]

            def load_xt_chunk(eng, c):
                cs = slice(c * NCT * QCHUNK, (c + 1) * NCT * QCHUNK)
                eng.dma_start(xtall[:, cs], xt_d[:, cs])

            qt_r = [qk_sb.tile([128, T], bf16, tag=f"qtr{p}", name=f"qtr{p}") for p in range(PAIRS)]
            kt_r = [qk_sb.tile([128, T], bf16, tag=f"ktr{p}", name=f"ktr{p}") for p in range(PAIRS)]
            v_ext = [qk_sb.tile([128, 4 * 128], bf16, tag=f"v{tt}", name=f"v{tt}") for tt in range(NT128)]
            att_out = [qk_sb.tile([128, T], bf16, tag=f"ao{p}", name=f"ao{p}") for p in range(PAIRS)]

            # ---- input DMAs: ALL on the scalar ring (ACT idle early) so
            # transfers serialize in need-order -- concurrent rings would
            # steal HBM bandwidth from the critical q0c0/k0c0 loads ----
            nc.scalar.dma_start(wqall[:, 0:C], wq_d[:, 0:C])  # pair-0 wq
            nc.scalar.dma_start(psh[:], psh_d[:])             # rope shift perm
            nc.scalar.dma_start(psw[:], psw_d[:])             # 64-half swap perm
            half = NCT * QCHUNK // 2
            nc.scalar.dma_start(xtall[:, 0:half], xt_d[:, 0:half])
            nc.scalar.dma_start(xtall[:, half : 2 * half], xt_d[:, half : 2 * half])
            nc.scalar.dma_start(wkall[:, 0:C], wk_d[:, 0:C])  # pair-0 wk
            nc.scalar.dma_start(cmap[:], cmap_d[:])
            nc.scalar.dma_start(smap[:], smap_d[:])
            nc.scalar.dma_start(wvall[:], wv_d[:])
            load_xt_chunk(nc.scalar, 1)
            nc.scalar.dma_start(wqall[:, C:], wq_d[:, C:])    # pair-1 wq
            nc.scalar.dma_start(wkall[:, C:], wk_d[:, C:])    # pair-1 wk
            load_xt_chunk(nc.scalar, 2)
            load_xt_chunk(nc.scalar, 3)
            nc.scalar.dma_start(woall[:], wo_d[:])

            # upper-triangular (incl. diagonal) keep-mask: tri[p, y] = p <= y
            nc.gpsimd.memset(tri[:], 1.0)
            nc.gpsimd.affine_select(
                out=tri[:],
                in_=tri[:],
                compare_op=mybir.AluOpType.is_ge,
                fill=0.0,
                base=0,
                pattern=[[1, 128]],
                channel_multiplier=-1,
            )
            tri_b = tri[:]  # broadcast view over 2 mask blocks built per-use

            # ---- PE warm-up: spin matmuls on tri during the input-DMA wait
            # so q0c0 runs at 2.4 GHz (HAM un-throttles after ~3.4us busy)
            wps = ps_fl.tile([128, 128], f32, tag="fl", name="ps_warm")
            for _ in range(46):
                nc.tensor.matmul(wps[:], lhsT=tri[:], rhs=tri[:], start=True, stop=True)

            # ---- work units -------------------------------------------------
            emitted = set()

            def qk_units(which, wall, p, c, dst, fast=False):
                """q or k chunk as 5 fine units: 4x(2 matmuls), then rope."""
                cs = slice(c * QCHUNK, (c + 1) * QCHUNK)
                st = {}

                def mms(u):
                    def fn():
                        if u == 0:
                            st["ps"] = ps_fl.tile([128, QCHUNK], f32, tag="fl", name="ps_qk")
                        for ci in (2 * u, 2 * u + 1):
                            nc.tensor.matmul(
                                st["ps"][:],
                                lhsT=w_ap(wall, ci, p),
                                rhs=xt_ap(ci, c * QCHUNK, (c + 1) * QCHUNK),
                                start=(ci == 0),
                                stop=(ci == NCT - 1),
                            )
                    return fn

                def rope():
                    raw = rope_tmp.tile([128, QCHUNK], bf16, tag="raw", name="raw")
                    nc.vector.tensor_copy(raw[:], st["ps"][:])
                    t1 = rope_tmp.tile([128, QCHUNK], bf16, tag="t1", name="t1")
                    nc.vector.tensor_mul(t1[:], raw[:], cmap[:, cs])
                    t2 = rope_tmp.tile([128, QCHUNK], bf16, tag="t2", name="t2")
                    # swap 32-row halves within each 64-row head block via a
                    # PE permutation matmul (DMA/GpSimd shifts both have
                    # us-scale latency or throughput problems)
                    shp = ps_fl.tile([128, QCHUNK], f32, tag="fl", name="ps_shf")
                    nc.tensor.matmul(
                        shp[:], lhsT=psh[:], rhs=raw[:], start=True, stop=True
                    )
                    nc.vector.tensor_mul(t2[:], shp[:], smap[:, cs])
                    nc.vector.tensor_add(dst[:, cs], t1[:], t2[:])
                    emitted.add((which, p, c))

                return [(450, mms(0)), (430, mms(1)), (430, mms(2)), (430, mms(3)),
                        (260 if fast else 60, rope)]

            def v_units(tt):
                """V for t-tile tt: 2x(4 matmuls) + interleave copies."""
                st = {}

                def mms(u):
                    def fn():
                        if u == 0:
                            nc.gpsimd.memset(v_ext[tt][:], 1.0)
                            st["ps"] = ps_fl.tile([128, M_CORE], f32, tag="fl", name="ps_v")
                        for ci in range(4 * u, 4 * u + 4):
                            nc.tensor.matmul(
                                st["ps"][:],
                                lhsT=xt_ap(ci, tt * 128, (tt + 1) * 128),
                                rhs=wv_ap(ci),
                                start=(ci == 0),
                                stop=(ci == NCT - 1),
                            )
                    return fn

                def copies():
                    vt, ps = v_ext[tt], st["ps"]
                    # dst layout: [V0 | 1 .. 1 | V1][V2 | 1 .. 1 | V3]
                    nc.vector.tensor_copy(vt[:, 0:64], ps[:, 0:64])
                    nc.vector.tensor_copy(vt[:, 192:320], ps[:, 64:192])
                    nc.vector.tensor_copy(vt[:, 448:512], ps[:, 192:256])
                    emitted.add(("v", tt))

                return [(460, mms(0)), (430, mms(1)), (60, copies)]

            def proj_units(qt, tailpool=False):
                """Output projection for q-tile qt: [128,1024] fp32->bf16->HBM."""
                st = {}
                # the last group drains after attention: the score-psum banks
                # are free then, doubling the number of concurrent proj chains
                pool, ptag = (ps_st, "st") if tailpool else (ps_fl, "fl")

                def half(jc):
                    def fn():
                        if jc == 0:
                            st["ob"] = out_sb.tile([128, C], bf16, tag="ob", name="ob")
                        ps = pool.tile([128, QCHUNK], f32, tag=ptag, name="ps_pj")
                        for p in range(PAIRS):
                            nc.tensor.matmul(
                                ps[:],
                                lhsT=att_out[p][:, qt * 128 : (qt + 1) * 128],
                                rhs=wo_ap(p, jc * QCHUNK, (jc + 1) * QCHUNK),
                                start=(p == 0),
                                stop=(p == PAIRS - 1),
                            )
                        nc.vector.tensor_copy(
                            st["ob"][:, jc * QCHUNK : (jc + 1) * QCHUNK], ps[:]
                        )
                        if jc == 1:
                            oeng = nc.sync if qt % 2 == 0 else nc.gpsimd
                            oeng.dma_start(
                                out_d[qt * 128 : (qt + 1) * 128, :], st["ob"][:]
                            )
                    return fn

                return [(460, half(0)), (460, half(1))]

            # ---- filler queue (self-balancing pop budget) ----
            fillq = deque()  # (pe_cost_ns, closure)
            debt = [0.0]
            pops_left = [68]  # pop points across all chunks (minus lean tail)

            proj_pending = [14720]  # cost of proj units not yet appended

            def pop_fill():
                queued = sum(c for c, _ in fillq) + proj_pending[0]
                budget = max(500.0, min(1500.0, queued / max(pops_left[0], 1)))
                pops_left[0] -= 1
                debt[0] += budget
                while fillq and debt[0] > 0:
                    cost, fn = fillq.popleft()
                    fn()
                    debt[0] -= cost

            def force_until(labels):
                while any(l not in emitted for l in labels):
                    assert fillq, f"missing prereqs {labels}"
                    cost, fn = fillq.popleft()
                    fn()
                    debt[0] -= cost

            # ---- attention chunk (inner loop software-pipelined by 1) ----
            def attn_chunk(p, j, next_qk=None, last=False):
                force_until([("q", p, j)] + [("k", p, c) for c in range(j + 1)])
                os2 = ps_os.tile([128, 2 * QCHUNK], f32, tag="os", name="ps_os")
                outA = os2[:, 0:QCHUNK]   # rows 0:64 attV_A, 64:128 sums_A
                outB = os2[:, QCHUNK:]    # rows 0:64 sums_B, 64:128 attV_B
                nkt = (j + 1) * (QCHUNK // KTILE)
                atts = [None] * nkt  # att2 tile + c0 per kb, for deferred AV

                def emit_scores(kb):
                    o = KTILE * kb - QCHUNK * j
                    c0 = max(o, 0)
                    qs = slice(j * QCHUNK + c0, (j + 1) * QCHUNK)
                    ks = slice(kb * KTILE, (kb + 1) * KTILE)
                    # both heads' scores in one 2-bank tile -> single exp;
                    # head B packed at column QCHUNK (not QCHUNK+c0) so the
                    # exp span [c0, 2*QCHUNK-c0) has no garbage columns
                    st2 = ps_st.tile([128, 2 * QCHUNK], f32, tag="st", name="ps_st")
                    nc.tensor.matmul(
                        st2[:, c0:QCHUNK],
                        lhsT=kt_r[p][0:64, ks],
                        rhs=qt_r[p][0:64, qs],
                        start=True,
                        stop=True,
                        tile_position=(0, 0),
                    )
                    nc.tensor.matmul(
                        st2[:, QCHUNK : 2 * QCHUNK - c0],
                        lhsT=kt_r[p][64:128, ks],
                        rhs=qt_r[p][64:128, qs],
                        start=True,
                        stop=True,
                        tile_position=(64, 0),
                    )
                    att2 = att_sb.tile([128, 2 * QCHUNK], bf16, tag="att", name="att2")
                    nc.scalar.activation(
                        att2[:, c0 : 2 * QCHUNK - c0],
                        st2[:, c0 : 2 * QCHUNK - c0],
                        Exp,
                        scale=0.125,
                    )
                    if o >= 0:  # diagonal tile: triangular mask, both heads
                        blk = QCHUNK - o
                        a = att2[:]
                        m_ap = bass.AP(a.tensor, a.offset + o, [list(a.ap[0]), [blk, 2], [1, 128]])
                        t_ap = bass.AP(tri_b.tensor, tri_b.offset, [list(tri_b.ap[0]), [0, 2], [1, 128]])
                        nc.vector.tensor_mul(m_ap, m_ap, t_ap)
                    atts[kb] = (att2, c0)

                def emit_av(kb):
                    att2, c0 = atts[kb]
                    atts[kb] = None
                    start = kb == 0
                    stop = kb == nkt - 1
                    blkA = slice((2 * p) * 128, (2 * p) * 128 + 128)
                    blkB = slice((2 * p + 1) * 128, (2 * p + 1) * 128 + 128)
                    nc.tensor.matmul(
                        outA[:, c0:],
                        lhsT=v_ext[kb][:, blkA],
                        rhs=att2[:, c0:QCHUNK],
                        start=start,
                        stop=stop,
                    )
                    nc.tensor.matmul(
                        outB[:, c0:],
                        lhsT=v_ext[kb][:, blkB],
                        rhs=att2[:, QCHUNK : 2 * QCHUNK - c0],
                        start=start,
                        stop=stop,
                    )

                for kb in range(nkt):
                    force_until([("v", kb)])
                    emit_scores(kb)
                    if kb == nkt // 2 and next_qk:
                        # prefetch next chunk's q/k so its rope latency
                        # hides under this chunk's exp stream
                        force_until(next_qk)
                    if kb > 0:
                        if not (last and kb > nkt - 3):
                            # keep the tail lean: no filler DVE work queued
                            # ahead of the final normalize chain
                            pop_fill()
                        emit_av(kb - 1)
                emit_av(nkt - 1)

                # release the accumulator early: one whole-tile cast to SBUF
                osb = misc_sb.tile([128, 2 * QCHUNK], bf16, tag="osb", name="osb")
                nc.vector.tensor_copy(osb[:], os2[:])
                oA = osb[:, 0:QCHUNK]
                oB = osb[:, QCHUNK:]
                cs = slice(j * QCHUNK, (j + 1) * QCHUNK)
                if last:
                    # latency-critical final normalize: 64-half swap on PE
                    # FIRST (all-finite bf16 operand -- recip-then-swap would
                    # feed inf junk lanes into the matmul and 0*inf=NaN),
                    # then reciprocal; junk lanes never enter any matmul
                    r0 = ps_fl.tile([128, QCHUNK], f32, tag="fl", name="ps_r0")
                    nc.tensor.matmul(r0[:], lhsT=psw[:], rhs=oA, start=True, stop=True)
                    r1 = ps_fl.tile([128, QCHUNK], f32, tag="fl", name="ps_r1")
                    nc.tensor.matmul(r1[:], lhsT=psw[:], rhs=oB, start=True, stop=True)
                    recA = misc_sb.tile([128, QCHUNK], f32, tag="recA", name="recA")
                    nc.vector.reciprocal_approx_fast(recA[:], r0[:])
                    recB = misc_sb.tile([128, QCHUNK], f32, tag="recB", name="recB")
                    nc.vector.reciprocal_approx_fast(recB[:], r1[:])
                    nc.vector.tensor_mul(att_out[p][0:64, cs], oA[0:64, :], recA[0:64, :])
                    nc.vector.tensor_mul(att_out[p][64:128, cs], oB[64:128, :], recB[64:128, :])
                    return
                # gather sums (aligned sub-partition copies), one reciprocal:
                # rows 0:64 = 1/sums_B, rows 64:128 = 1/sums_A
                sc = misc_sb.tile([128, QCHUNK], f32, tag="sc", name="sums_sb")
                nc.vector.tensor_copy(sc[0:64, :], oB[0:64, :])
                nc.vector.tensor_copy(sc[64:128, :], oA[64:128, :])
                rec_raw = misc_sb.tile([128, QCHUNK], f32, tag="rec_raw", name="rec_raw")
                nc.vector.reciprocal_approx_fast(rec_raw[:], sc[:])
                # swap halves so divisors align with their heads' rows
                rec = misc_sb.tile([128, QCHUNK], f32, tag="rec", name="rec")
                deng = nc.gpsimd if p == 0 else nc.sync
                deng.dma_start(rec[0:64, :], rec_raw[64:128, :])
                deng.dma_start(rec[64:128, :], rec_raw[0:64, :])
                nc.vector.tensor_mul(att_out[p][0:64, cs], oA[0:64, :], rec[0:64, :])
                nc.vector.tensor_mul(att_out[p][64:128, cs], oB[64:128, :], rec[64:128, :])

            # ---- prologue: minimum needed for attn(0,0) ----
            for _cost, fn in (
                qk_units("q", wqall, 0, 0, qt_r[0], fast=True)
                + qk_units("k", wkall, 0, 0, kt_r[0], fast=True)
                + v_units(0)
            ):
                fn()

            # ---- queue the rest, in exact deadline order:
            # V tiles of group c are needed (per-kb) before chunk (p,c) ends;
            # q/k of (1,c) at (1,c) start; q/k of (0,c+1) at (0,c+1) start --
            # which precedes (0,c+1)'s V group ----
            for tt in (1, 2, 3):
                fillq.extend(v_units(tt))
            fillq.extend(qk_units("q", wqall, 1, 0, qt_r[1], fast=True))
            fillq.extend(qk_units("k", wkall, 1, 0, kt_r[1], fast=True))
            for c in range(1, NQC):
                fillq.extend(qk_units("q", wqall, 0, c, qt_r[0], fast=True))
                fillq.extend(qk_units("k", wkall, 0, c, kt_r[0], fast=True))
                for tt in range(4 * c, 4 * c + 4):
                    fillq.extend(v_units(tt))
                fillq.extend(qk_units("q", wqall, 1, c, qt_r[1], fast=True))
                fillq.extend(qk_units("k", wkall, 1, c, kt_r[1], fast=True))

            # ---- main pipeline: alternate pairs so chunk boundaries of one
            # pair overlap the other pair's independent attention work ----
            order = [(p, j) for j in range(NQC) for p in range(PAIRS)]
            for idx, (p, j) in enumerate(order):
                nxt = order[idx + 1] if idx + 1 < len(order) else None
                next_qk = (
                    [("q", nxt[0], nxt[1]), ("k", nxt[0], nxt[1])] if nxt else None
                )
                attn_chunk(p, j, next_qk, last=(nxt is None))
                if p == 1:
                    for qt in range(4 * j, 4 * j + 4):
                        fillq.extend(proj_units(qt, tailpool=(j == NQC - 1)))
                    proj_pending[0] -= 3680
            while fillq:
                _c, fn = fillq.popleft()
                fn()

    nc.compile()
    return nc


def _prep_inputs(x, Wq, Wk, Wv, Wo, cos, sin):
    """Host-side sharding + layout prep. Returns list of per-core in_maps."""
    x = np.asarray(x, np.float32)
    Wq, Wk, Wv, Wo = (np.asarray(w, np.float32) for w in (Wq, Wk, Wv, Wo))
    cos, sin = np.asarray(cos, np.float32), np.asarray(sin, np.float32)

    # permute W rows to [evens; odds] within each head (rope pairing -> +-32)
    perm = np.concatenate(
        [
            np.concatenate(
                [np.arange(h * HD, (h + 1) * HD, 2), np.arange(h * HD + 1, (h + 1) * HD, 2)]
            )
            for h in range(H)
        ]
    )
    Wqp = Wq[perm]
    Wkp = Wk[perm]

    # rope maps [128, T] (identical for both heads of a pair, all cores)
    cosT = cos.T  # [32, T]
    sinT = sin.T
    cmap = np.empty((128, T), np.float32)
    smap = np.empty((128, T), np.float32)
    for blk in range(4):
        cmap[blk * 32 : (blk + 1) * 32] = cosT
        smap[blk * 32 : (blk + 1) * 32] = sinT if blk % 2 else -sinT
    cmap = cmap.astype(_bf16)
    smap = smap.astype(_bf16)

    # device layouts are [128, *] with big contiguous per-partition lines:
    # xt: [128, c(4) x ci(8) x 512]; wq/wk/wv: [128, ci(8) x 256];
    # wo: [128, pair(2) x 1024]
    def shuf_xt(xT):  # xT [C, T]
        v = xT.reshape(NCT, 128, NQC, QCHUNK)  # (ci, p, c, u)
        return np.ascontiguousarray(
            v.transpose(1, 2, 0, 3).reshape(128, NQC * NCT * QCHUNK)
        ).astype(_bf16)

    def shuf_w(wT):  # wT [C, M_CORE] -> [128, ci(8) x 256] (ci-major, for wv)
        v = wT.reshape(NCT, 128, M_CORE)  # (ci, p, v)
        return np.ascontiguousarray(
            v.transpose(1, 0, 2).reshape(128, NCT * M_CORE)
        ).astype(_bf16)

    def shuf_w_pair(wT):  # wT [C, M_CORE] -> [128, pair(2) x ci(8) x 128]
        v = wT.reshape(NCT, 128, PAIRS, 128)  # (ci, p, pair, col)
        return np.ascontiguousarray(
            v.transpose(1, 2, 0, 3).reshape(128, NCT * M_CORE)
        ).astype(_bf16)

    def shuf_wo(woT):  # woT [M_CORE, C]
        v = woT.reshape(PAIRS, 128, C)  # (pair, p, v)
        return np.ascontiguousarray(
            v.transpose(1, 0, 2).reshape(128, PAIRS * C)
        ).astype(_bf16)

    xTb = [shuf_xt(x[b].T) for b in range(B)]

    # rope shift permutation: out row j = in row j^32 (32-row half swap
    # within each 64-row head block), applied on PE as out = P.T @ raw
    psh = np.zeros((128, 128), np.float32)
    psh[np.arange(128), np.arange(128) ^ 32] = 1.0
    psh = psh.astype(_bf16)
    psw = np.zeros((128, 128), np.float32)
    psw[np.arange(128), np.arange(128) ^ 64] = 1.0
    psw = psw.astype(_bf16)

    in_maps = []
    for core in range(N_CORES):
        b, g = divmod(core, GROUPS)
        ms = slice(g * M_CORE, (g + 1) * M_CORE)
        in_maps.append(
            {
                "xt": xTb[b],
                "wqt": shuf_w_pair(Wqp[ms].T),
                "wkt": shuf_w_pair(Wkp[ms].T),
                "wvt": shuf_w(Wv[ms].T),
                "wot": shuf_wo(Wo[:, ms].T),
                "cmap": cmap,
                "smap": smap,
                "pshift": psh,
                "pswap": psw,
            }
        )
    return in_maps


def _ensure_ntff_hook():
    """Install an antenv.axon_hooks shim so trace=True works in this
    container (the image's antenv lacks the axon_hooks module)."""
    import sys
    import types

    try:
        from antenv.axon_hooks import get_axon_ntff_profile_hook  # noqa: F401

        return
    except ImportError:
        pass
    sys.path.insert(0, "/root/.axon_site")
    from trn_agent_boot.trn_boot import _ntff_profile_via_ctypes

    hook = _ntff_profile_via_ctypes("/opt/axon/libaxon_pjrt.so")
    mod = types.ModuleType("antenv.axon_hooks")
    mod._hook = hook
    mod.get_axon_ntff_profile_hook = lambda: mod._hook
    mod.set_axon_ntff_profile_hook = lambda h: setattr(mod, "_hook", h)
    sys.modules["antenv.axon_hooks"] = mod

    # no bucket creds in this container; keep artifacts local
    import concourse.bass_utils as bu

    bu.upload_artifacts = lambda tmpdir: tmpdir


def kernel(x, Wq, Wk, Wv, Wo, cos, sin):
    global LAST_RESULTS
    from concourse.bass_utils import run_bass_kernel_spmd

    if "nc" not in _CACHE:
        _CACHE["nc"] = _build_bass()
    nc = _CACHE["nc"]

    in_maps = _prep_inputs(x, Wq, Wk, Wv, Wo, cos, sin)
    trace = bool(int(os.environ.get("KERNEL_TRACE", "0")))
    if trace:
        _ensure_ntff_hook()
    res = run_bass_kernel_spmd(
        nc, in_maps, core_ids=list(range(N_CORES)), trace=trace
    )
    LAST_RESULTS = res

    out = np.zeros((B, T, C), np.float32)
    for core in range(N_CORES):
        b = core // GROUPS
        out[b] += res.results[core]["out"].astype(np.float32)
    return out
